# revision 11
# baseline (speedup 1.0000x reference)
"""GNN message-passing layer on 8 Trainium2 NeuronCores.

Strategy: receiver-range sharding. Core c owns nodes [c*12800, (c+1)*12800)
and receives exactly the edges whose receiver falls in its range, so each core
computes its full output slice with no cross-core collectives.

Host-side layout per core:
  - nodes padded to 102400 = 800 windows of 128; each core owns 100 windows
  - each window's edges are grouped by sender quarter (node-id // 25600, so
    quarter-local ids fit dma_gather's int16 index limit), each (window,
    quarter) group padded to a fixed 640 slots (5 tiles of 128)
  - slot order: [batch of B windows][quarter][window-in-batch][640]

Device pipeline (Tile framework, fully static):
  - senders: dma_gather (non-transpose mode, bf16, 256B rows) from the
    padded node table, one call per quarter per batch, spread over 4 SWDGE
    queues (transpose-mode gathers share the xbar and corrupt when run
    concurrently; non-transpose CME copies are concurrency-safe)
  - one batched 3D dma_start_transpose (HWDGE xbar) per batch converts the
    gathered edge-major [128, k, 128] tile to feature-major
  - edge features + ones row then overwrite the spare partitions 64:98, so
    h_pre = XT.T @ W1mod  is one ldw+matmul
    (W1mod rows: 0:64 sender W1, 64:96 edge W1, 96 b1)
  - receivers are window-local, so no gather: PrW = nodes_w @ W1[64:128] is
    computed once per 128-node window, and per-edge receiver contributions
    are injected via  h_pre += maskT.T @ PrW  where maskT[n,e] = (recv_e == n)
  - masks are built on DVE in batched 640-wide ops, one pair per (window,
    quarter) block: maskT via tensor_scalar is_equal against a per-partition
    iota, mask via tensor_tensor is_equal of a recv-id broadcast (step-0 AP)
    against a tiled iota table
  - h = relu(h_pre) with one 640-wide ACT op per block (h_ps spans 2 PSUM
    banks; each matmul writes a 128-col slice within one bank), then window
    aggregate  aggT += h.T @ mask  accumulated in PSUM over 20 tiles
  - window epilogue: out = (aggT.T @ W2) * inv_deg + nodes@Wn + gate*b2 + bn
    via two matmuls into one PSUM bank and a single DVE scalar_tensor_tensor
    (host precomputes inv_deg = 1/max(deg,1), gate = deg>0)
"""
import numpy as np
import ml_dtypes
from contextlib import ExitStack

import concourse.bass as bass
import concourse.tile as tile
from concourse import bacc, mybir
import concourse.bass_utils as bass_utils

BF16 = mybir.dt.bfloat16
F32 = mybir.dt.float32
I16 = mybir.dt.int16
U8 = mybir.dt.uint8
bfnp = ml_dtypes.bfloat16

# problem shapes (hardcoded per harness contract)
N_NODES = 100000
N_EDGES = 1600000
NODE_F = 64
EDGE_F = 32
OUT_F = 64
HIDDEN = 128

NCORES = 8
NODES_PAD = 102400            # 800 windows of 128
W_TOTAL = NODES_PAD // 128    # 800
W_CORE = W_TOTAL // NCORES    # 100 windows per core
NODES_CORE = W_CORE * 128     # 12800
QUARTER = NODES_PAD // 4      # 25600 (< int16 max)
B = 5                         # windows per batch
NBATCH = W_CORE // B          # 20

_cache = {}


def _build_program(slots_wq: int):
    """Build + compile the (single, SPMD-shared) Bass program."""
    slots_w = 4 * slots_wq            # slots per window
    tiles_w = slots_w // 128          # tiles per window
    tiles_wq = slots_wq // 128        # tiles per (window, quarter)
    slots_b = B * slots_w             # slots per batch
    slots_core = W_CORE * slots_w
    tiles_core = slots_core // 128

    nc = bacc.Bacc("TRN2", target_bir_lowering=False, debug=False,
                   enable_asserts=False, num_devices=NCORES,
                   num_swdge_queues=4)

    tbl_s = nc.dram_tensor("tbl_s", [NODES_PAD, 128], BF16, kind="ExternalInput")
    edges_t = nc.dram_tensor("edges_t", [34, slots_core], BF16, kind="ExternalInput")
    sidx = nc.dram_tensor("sidx", [128, slots_core // 16], I16, kind="ExternalInput")
    recvb = nc.dram_tensor("recvb", [128, tiles_core], BF16, kind="ExternalInput")
    recvf = nc.dram_tensor("recvf", [1, slots_core], BF16, kind="ExternalInput")
    nodes_t = nc.dram_tensor("nodes_t", [66, NODES_CORE], F32, kind="ExternalInput")
    invdeg = nc.dram_tensor("invdeg", [128, W_CORE], F32, kind="ExternalInput")
    w1mod = nc.dram_tensor("w1mod", [128, HIDDEN], BF16, kind="ExternalInput")
    w1r = nc.dram_tensor("w1r", [NODE_F, HIDDEN], F32, kind="ExternalInput")
    w2 = nc.dram_tensor("w2", [HIDDEN, OUT_F], BF16, kind="ExternalInput")
    waug = nc.dram_tensor("waug", [66, OUT_F], F32, kind="ExternalInput")
    iota = nc.dram_tensor("iota", [128, slots_wq], BF16, kind="ExternalInput")
    iotap = nc.dram_tensor("iotap", [128, 1], F32, kind="ExternalInput")
    iotap8 = nc.dram_tensor("iotap8", [128, 1], U8, kind="ExternalInput")
    out_d = nc.dram_tensor("out", [NODES_CORE, OUT_F], F32, kind="ExternalOutput")

    relu = mybir.ActivationFunctionType.Relu
    cpy = mybir.ActivationFunctionType.Copy
    iseq = mybir.AluOpType.is_equal

    with tile.TileContext(nc) as tc:
        with ExitStack() as ctx:
            cpool = ctx.enter_context(tc.tile_pool(name="const", bufs=1))
            bpool = ctx.enter_context(tc.tile_pool(name="batch", bufs=2))
            spool = ctx.enter_context(tc.tile_pool(name="small", bufs=4))
            opool = ctx.enter_context(tc.tile_pool(name="outs", bufs=3))
            ph = ctx.enter_context(tc.tile_pool(name="ph", bufs=2, space="PSUM"))
            pagg = ctx.enter_context(tc.tile_pool(name="pagg", bufs=2, space="PSUM"))
            pprw = ctx.enter_context(tc.tile_pool(name="pprw", bufs=1, space="PSUM"))
            pout = ctx.enter_context(tc.tile_pool(name="pout", bufs=1, space="PSUM"))

            w1mod_t = cpool.tile([128, HIDDEN], BF16)
            nc.sync.dma_start(w1mod_t[:], w1mod.ap())
            w1r_t = cpool.tile([NODE_F, HIDDEN], F32)
            nc.sync.dma_start(w1r_t[:], w1r.ap())
            w2_t = cpool.tile([HIDDEN, OUT_F], BF16)
            nc.sync.dma_start(w2_t[:], w2.ap())
            waug_t = cpool.tile([66, OUT_F], F32)
            nc.sync.dma_start(waug_t[:], waug.ap())
            iota_t = cpool.tile([128, slots_wq], BF16)
            nc.sync.dma_start(iota_t[:], iota.ap())
            iotap_t = cpool.tile([128, 1], F32)
            nc.sync.dma_start(iotap_t[:], iotap.ap())
            iotap8_t = cpool.tile([128, 1], U8)
            nc.sync.dma_start(iotap8_t[:], iotap8.ap())
            invdeg_t = cpool.tile([128, W_CORE], F32)
            nc.sync.dma_start(invdeg_t[:], invdeg.ap())

            for b in range(NBATCH):
                s0 = b * slots_b                      # batch slot base
                em = bpool.tile([128, slots_b], BF16, tag="em")
                st = bpool.tile([128, slots_b], BF16, tag="st")
                rT = bpool.tile([128, slots_b], BF16, tag="rT")
                sidx_t = bpool.tile([128, slots_b // 16], I16, tag="sidx")
                recvb_t = bpool.tile([128, slots_b // 128], BF16, tag="recvb")
                nodesb_t = bpool.tile([66, B * 128], F32, tag="nodesb")

                nc.sync.dma_start(sidx_t[:],
                                  sidx.ap()[:, s0 // 16:(s0 + slots_b) // 16])
                nc.sync.dma_start(recvb_t[:],
                                  recvb.ap()[:, b * B * tiles_w:(b + 1) * B * tiles_w])
                nc.sync.dma_start(nodesb_t[:],
                                  nodes_t.ap()[:, b * B * 128:(b + 1) * B * 128])
                # partition-broadcast of window-local receiver ids
                nc.sync.dma_start(
                    rT[:], recvf.ap()[0:1, s0:s0 + slots_b].to_broadcast(
                        [128, slots_b]))

                # sender gathers: one per quarter, spread over 4 SWDGE queues
                qs = B * slots_wq                     # slots per quarter in batch
                for q in range(4):
                    nc.gpsimd.dma_gather(
                        out_ap=em[:, q * qs:(q + 1) * qs]
                        .rearrange("p (c f) -> p c f", f=128),
                        in_ap=tbl_s.ap()[q * QUARTER:(q + 1) * QUARTER, :],
                        idxs_ap=sidx_t[:, q * qs // 16:(q + 1) * qs // 16],
                        num_idxs=qs, num_idxs_reg=qs, elem_size=128,
                        transpose=False, single_packet=False, queue_num=q,
                    )
                # per-quarter transpose + edge-feature overwrite: lets
                # quarter q's compute start while quarter q+1 still gathers
                for q in range(4):
                    r0, r1 = q * qs, (q + 1) * qs
                    nc.sync.dma_start(
                        out=st[:, r0:r1].rearrange("p (k f) -> p k f", f=128),
                        in_=em[:, r0:r1].rearrange("p (k f) -> p k f", f=128),
                        transpose=True)
                    nc.sync.dma_start(st[64:98, r0:r1],
                                      edges_t.ap()[:, s0 + r0:s0 + r1])

                for wi in range(B):
                    wg = b * B + wi                   # global window index
                    # receiver projection for this window's 128 nodes
                    prw_ps = pprw.tile([128, HIDDEN], F32, tag="prw")
                    nc.tensor.matmul(
                        out=prw_ps[:],
                        lhsT=nodesb_t[0:NODE_F, wi * 128:(wi + 1) * 128],
                        rhs=w1r_t[:], start=True, stop=True)
                    prw_s = spool.tile([128, HIDDEN], BF16, tag="prw_s")
                    nc.scalar.activation(prw_s[:], prw_ps[:], cpy)

                    agg_ps = pagg.tile([128, 128], F32, tag="agg")
                    for q in range(4):
                        # quarter-block of tiles_wq tiles (slots_wq slots)
                        off = q * qs + wi * slots_wq
                        tcol = off // 128
                        mask_b = spool.tile([128, slots_wq], BF16, tag="mask")
                        nc.vector.tensor_tensor(
                            out=mask_b[:].rearrange("p (c f) -> p c f", f=128),
                            in0=recvb_t[:, tcol:tcol + tiles_wq]
                            .to_broadcast([128, tiles_wq, 128]),
                            in1=iota_t[:].rearrange("p (c f) -> p c f", f=128),
                            op=iseq)
                        maskT_b = spool.tile([128, slots_wq], BF16, tag="maskT")
                        nc.vector.tensor_scalar(
                            out=maskT_b[:], in0=rT[:, off:off + slots_wq],
                            scalar1=iotap_t[:], scalar2=None, op0=iseq)
                        h_ps = ph.tile([128, slots_wq], F32, tag="h")
                        for j in range(tiles_wq):
                            so = off + j * 128
                            hsl = h_ps[:, j * 128:(j + 1) * 128]
                            nc.tensor.matmul(out=hsl, lhsT=st[:, so:so + 128],
                                             rhs=w1mod_t[:], start=True,
                                             stop=False)
                            nc.tensor.matmul(
                                out=hsl,
                                lhsT=maskT_b[:, j * 128:(j + 1) * 128],
                                rhs=prw_s[:], start=False, stop=True)
                        h_s = spool.tile([128, slots_wq], BF16, tag="hs")
                        nc.scalar.activation(h_s[:], h_ps[:], relu)
                        for j in range(tiles_wq):
                            nc.tensor.matmul(
                                out=agg_ps[:],
                                lhsT=h_s[:, j * 128:(j + 1) * 128],
                                rhs=mask_b[:, j * 128:(j + 1) * 128],
                                start=(q == 0 and j == 0),
                                stop=(q == 3 and j == tiles_wq - 1))
                    # window epilogue: out = (aggT.T@W2)*invdeg + nodes@waug
                    agg_s = opool.tile([128, 128], BF16, tag="aggs")
                    nc.scalar.activation(agg_s[:], agg_ps[:], cpy)
                    ot_ps = pout.tile([128, 2 * OUT_F], F32, tag="ot")
                    nc.tensor.matmul(out=ot_ps[:, 0:OUT_F], lhsT=agg_s[:],
                                     rhs=w2_t[:], start=True, stop=True)
                    nc.tensor.matmul(out=ot_ps[:, OUT_F:2 * OUT_F],
                                     lhsT=nodesb_t[:, wi * 128:(wi + 1) * 128],
                                     rhs=waug_t[:], start=True, stop=True)
                    t1 = opool.tile([128, OUT_F], F32, tag="t1")
                    nc.vector.tensor_scalar(
                        out=t1[:], in0=ot_ps[:, 0:OUT_F],
                        scalar1=invdeg_t[:, wg:wg + 1],
                        scalar2=None, op0=mybir.AluOpType.mult)
                    ot = opool.tile([128, OUT_F], F32, tag="otf")
                    nc.vector.tensor_add(ot[:], t1[:], ot_ps[:, OUT_F:2 * OUT_F])
                    nc.sync.dma_start(out_d.ap()[wg * 128:(wg + 1) * 128, :], ot[:])

    nc.compile()
    return nc


def _prep_inputs(nodes, edges, senders, receivers, W1, b1, W2, b2, Wn, bn,
                 slots_wq):
    """Host-side data layout. Returns per-core in_maps."""
    slots_w = 4 * slots_wq
    slots_core = W_CORE * slots_w

    nodes_pad = np.zeros((NODES_PAD, NODE_F), np.float32)
    nodes_pad[:N_NODES] = nodes

    tbl_s = np.zeros((NODES_PAD, 128), bfnp)
    tbl_s[:, :NODE_F] = nodes_pad.astype(bfnp)

    deg = np.bincount(receivers, minlength=NODES_PAD).astype(np.float32)
    invdeg_full = 1.0 / np.maximum(deg, 1.0)
    gate_full = (deg > 0).astype(np.float32)

    # shared weight tensors
    w1mod = np.zeros((128, HIDDEN), bfnp)
    w1mod[:NODE_F] = W1[:NODE_F].astype(bfnp)
    w1mod[NODE_F:NODE_F + EDGE_F] = W1[2 * NODE_F:].astype(bfnp)
    w1mod[NODE_F + EDGE_F] = b1.astype(bfnp)
    w1r = np.ascontiguousarray(W1[NODE_F:2 * NODE_F]).astype(np.float32)
    w2b = W2.astype(bfnp)
    waug = np.zeros((66, OUT_F), np.float32)
    waug[:NODE_F] = Wn
    waug[NODE_F] = b2
    waug[NODE_F + 1] = bn
    iota_b = np.tile(np.arange(128, dtype=np.float32), (128, slots_wq // 128)
                     ).astype(bfnp)
    iotap = np.arange(128, dtype=np.float32).reshape(128, 1)
    iotap8 = np.arange(128, dtype=np.uint8).reshape(128, 1)

    core_of_edge = receivers // NODES_CORE
    in_maps = []
    for c in range(NCORES):
        lo = c * NODES_CORE
        eid = np.nonzero(core_of_edge == c)[0]
        rloc = receivers[eid] - lo
        w_loc = rloc >> 7
        q = senders[eid] // QUARTER
        # sender as fastest key: each (window, quarter) group's gather reads
        # the node table in ascending order (HBM row-buffer locality)
        order = np.lexsort((senders[eid], q, w_loc))
        eid, rloc, w_loc, q = eid[order], rloc[order], w_loc[order], q[order]
        grp = w_loc * 4 + q
        counts = np.bincount(grp, minlength=W_CORE * 4)
        assert counts.max() <= slots_wq, f"quarter run {counts.max()} > {slots_wq}"
        starts = np.searchsorted(grp, np.arange(W_CORE * 4))
        pos = np.arange(len(eid)) - starts[grp]
        base_wq = ((w_loc // B) * (B * slots_w) + q * (B * slots_wq)
                   + (w_loc % B) * slots_wq)
        slot = base_wq + pos

        sidx_f = np.zeros(slots_core, np.int16)
        sidx_f[slot] = (senders[eid] % QUARTER).astype(np.int16)
        recvw = np.full(slots_core, 200.0, np.float32)
        recvw[slot] = (rloc & 127).astype(np.float32)
        edges_t = np.zeros((34, slots_core), bfnp)
        edges_t[:EDGE_F, slot] = edges[eid].T.astype(bfnp)
        edges_t[EDGE_F, slot] = 1.0

        nodes_taug = np.zeros((66, NODES_CORE), np.float32)
        nodes_taug[:NODE_F] = nodes_pad[lo:lo + NODES_CORE].T
        nodes_taug[NODE_F] = gate_full[lo:lo + NODES_CORE]
        nodes_taug[NODE_F + 1] = 1.0

        in_maps.append({
            "tbl_s": tbl_s,
            "edges_t": edges_t,
            "sidx": np.tile(sidx_f.reshape(-1, 16).T, (8, 1)),
            "recvb": recvw.astype(bfnp).reshape(-1, 128).T.copy(),
            "recvf": recvw.astype(bfnp).reshape(1, -1),
            "nodes_t": nodes_taug,
            "invdeg": invdeg_full[lo:lo + NODES_CORE].reshape(-1, 128).T.copy(),
            "w1mod": w1mod, "w1r": w1r, "w2": w2b, "waug": waug,
            "iota": iota_b, "iotap": iotap, "iotap8": iotap8,
        })
    return in_maps


def kernel(nodes, edges, senders, receivers, W1, b1, W2, b2, Wn, bn,
           _trace=False):
    senders = np.asarray(senders).astype(np.int64)
    receivers = np.asarray(receivers).astype(np.int64)
    nodes = np.asarray(nodes, np.float32)
    edges = np.asarray(edges, np.float32)

    # fixed quarter-run capacity; recompile only if data exceeds it
    slots_wq = 640
    cnt = np.bincount(
        (receivers // NODES_CORE) * (W_CORE * 4)
        + (((receivers % NODES_CORE) >> 7) * 4) + senders // QUARTER,
        minlength=NCORES * W_CORE * 4).max()
    while cnt > slots_wq:
        slots_wq += 128

    if slots_wq not in _cache:
        _cache[slots_wq] = _build_program(slots_wq)
    nc = _cache[slots_wq]

    in_maps = _prep_inputs(nodes, edges, senders, receivers,
                           np.asarray(W1, np.float32), np.asarray(b1, np.float32),
                           np.asarray(W2, np.float32), np.asarray(b2, np.float32),
                           np.asarray(Wn, np.float32), np.asarray(bn, np.float32),
                           slots_wq)

    res = bass_utils.run_bass_kernel_spmd(
        nc, in_maps, core_ids=list(range(NCORES)), trace=_trace)

    out = np.concatenate([res.results[c]["out"] for c in range(NCORES)], axis=0)
    kernel.last_results = res
    return out[:N_NODES]


# revision 12
# speedup vs baseline: 1.3048x; 1.3048x over previous
"""GNN message-passing layer on 8 Trainium2 NeuronCores.

Strategy: receiver-range sharding. Core c owns nodes [c*12800, (c+1)*12800)
and receives exactly the edges whose receiver falls in its range, so each core
computes its full output slice with no cross-core collectives.

Host-side layout per core:
  - nodes padded to 102400 = 800 windows of 128; each core owns 100 windows
  - each window's edges are grouped by sender quarter (node-id // 25600, so
    quarter-local ids fit dma_gather's int16 index limit), each (window,
    quarter) group padded to a fixed 640 slots (5 tiles of 128)
  - slot order: [batch of B windows][quarter][window-in-batch][640]

Device pipeline (Tile framework, fully static):
  - senders: dma_gather (non-transpose mode, bf16, 256B rows) from the
    padded node table, one call per quarter per batch, spread over 4 SWDGE
    queues (transpose-mode gathers share the xbar and corrupt when run
    concurrently; non-transpose CME copies are concurrency-safe)
  - one batched 3D dma_start_transpose (HWDGE xbar) per batch converts the
    gathered edge-major [128, k, 128] tile to feature-major
  - edge features + ones row then overwrite the spare partitions 64:98, so
    h_pre = XT.T @ W1mod  is one ldw+matmul
    (W1mod rows: 0:64 sender W1, 64:96 edge W1, 96 b1)
  - receivers are window-local, so no gather: PrW = nodes_w @ W1[64:128] is
    computed once per 128-node window, and per-edge receiver contributions
    are injected via  h_pre += maskT.T @ PrW  where maskT[n,e] = (recv_e == n)
  - masks are built on DVE in batched 640-wide ops, one pair per (window,
    quarter) block: maskT via tensor_scalar is_equal against a per-partition
    iota, mask via tensor_tensor is_equal of a recv-id broadcast (step-0 AP)
    against a tiled iota table
  - h = relu(h_pre) with one 640-wide ACT op per block (h_ps spans 2 PSUM
    banks; each matmul writes a 128-col slice within one bank), then window
    aggregate  aggT += h.T @ mask  accumulated in PSUM over 20 tiles
  - window epilogue: out = (aggT.T @ W2) * inv_deg + nodes@Wn + gate*b2 + bn
    via two matmuls into one PSUM bank and a single DVE scalar_tensor_tensor
    (host precomputes inv_deg = 1/max(deg,1), gate = deg>0)
"""
import numpy as np
import ml_dtypes
from contextlib import ExitStack

import concourse.bass as bass
import concourse.tile as tile
from concourse import bacc, mybir
import concourse.bass_utils as bass_utils

BF16 = mybir.dt.bfloat16
F32 = mybir.dt.float32
I16 = mybir.dt.int16
U8 = mybir.dt.uint8
bfnp = ml_dtypes.bfloat16

# problem shapes (hardcoded per harness contract)
N_NODES = 100000
N_EDGES = 1600000
NODE_F = 64
EDGE_F = 32
OUT_F = 64
HIDDEN = 128

NCORES = 8
NODES_PAD = 102400            # 800 windows of 128
W_TOTAL = NODES_PAD // 128    # 800
W_CORE = W_TOTAL // NCORES    # 100 windows per core
NODES_CORE = W_CORE * 128     # 12800
QUARTER = NODES_PAD // 4      # 25600 (< int16 max)
B = 5                         # windows per batch
NBATCH = W_CORE // B          # 20

_cache = {}


def _build_program(slots_wq: int):
    """Build + compile the (single, SPMD-shared) Bass program."""
    slots_w = 4 * slots_wq            # slots per window
    tiles_w = slots_w // 128          # tiles per window
    tiles_wq = slots_wq // 128        # tiles per (window, quarter)
    slots_b = B * slots_w             # slots per batch
    slots_core = W_CORE * slots_w
    tiles_core = slots_core // 128

    nc = bacc.Bacc("TRN2", target_bir_lowering=False, debug=False,
                   enable_asserts=False, num_devices=NCORES,
                   num_swdge_queues=4)

    tbl_s = nc.dram_tensor("tbl_s", [NODES_PAD, 128], BF16, kind="ExternalInput")
    edges_t = nc.dram_tensor("edges_t", [34, slots_core], BF16, kind="ExternalInput")
    sidx = nc.dram_tensor("sidx", [128, slots_core // 16], I16, kind="ExternalInput")
    recvb = nc.dram_tensor("recvb", [128, tiles_core], BF16, kind="ExternalInput")
    recvf = nc.dram_tensor("recvf", [1, slots_core], BF16, kind="ExternalInput")
    nodes_t = nc.dram_tensor("nodes_t", [66, NODES_CORE], F32, kind="ExternalInput")
    invdeg = nc.dram_tensor("invdeg", [128, W_CORE], F32, kind="ExternalInput")
    w1mod = nc.dram_tensor("w1mod", [128, HIDDEN], BF16, kind="ExternalInput")
    w1r = nc.dram_tensor("w1r", [NODE_F, HIDDEN], F32, kind="ExternalInput")
    w2 = nc.dram_tensor("w2", [HIDDEN, OUT_F], BF16, kind="ExternalInput")
    waug = nc.dram_tensor("waug", [66, OUT_F], F32, kind="ExternalInput")
    iota = nc.dram_tensor("iota", [128, slots_wq], BF16, kind="ExternalInput")
    iotap = nc.dram_tensor("iotap", [128, 1], F32, kind="ExternalInput")
    iotap8 = nc.dram_tensor("iotap8", [128, 1], U8, kind="ExternalInput")
    out_d = nc.dram_tensor("out", [NODES_CORE, OUT_F], F32, kind="ExternalOutput")

    relu = mybir.ActivationFunctionType.Relu
    cpy = mybir.ActivationFunctionType.Copy
    iseq = mybir.AluOpType.is_equal

    with tile.TileContext(nc) as tc:
        with ExitStack() as ctx:
            cpool = ctx.enter_context(tc.tile_pool(name="const", bufs=1))
            bpool = ctx.enter_context(tc.tile_pool(name="batch", bufs=2))
            spool = ctx.enter_context(tc.tile_pool(name="small", bufs=4))
            opool = ctx.enter_context(tc.tile_pool(name="outs", bufs=3))
            ph = ctx.enter_context(tc.tile_pool(name="ph", bufs=2, space="PSUM"))
            pagg = ctx.enter_context(tc.tile_pool(name="pagg", bufs=2, space="PSUM"))
            pprw = ctx.enter_context(tc.tile_pool(name="pprw", bufs=1, space="PSUM"))
            pout = ctx.enter_context(tc.tile_pool(name="pout", bufs=1, space="PSUM"))

            w1mod_t = cpool.tile([128, HIDDEN], BF16)
            nc.sync.dma_start(w1mod_t[:], w1mod.ap())
            w1r_t = cpool.tile([NODE_F, HIDDEN], F32)
            nc.sync.dma_start(w1r_t[:], w1r.ap())
            w2_t = cpool.tile([HIDDEN, OUT_F], BF16)
            nc.sync.dma_start(w2_t[:], w2.ap())
            waug_t = cpool.tile([66, OUT_F], F32)
            nc.sync.dma_start(waug_t[:], waug.ap())
            iota_t = cpool.tile([128, slots_wq], BF16)
            nc.sync.dma_start(iota_t[:], iota.ap())
            iotap_t = cpool.tile([128, 1], F32)
            nc.sync.dma_start(iotap_t[:], iotap.ap())
            iotap8_t = cpool.tile([128, 1], U8)
            nc.sync.dma_start(iotap8_t[:], iotap8.ap())
            invdeg_t = cpool.tile([128, W_CORE], F32)
            nc.sync.dma_start(invdeg_t[:], invdeg.ap())

            for b in range(NBATCH):
                s0 = b * slots_b                      # batch slot base
                em = bpool.tile([128, slots_b], BF16, tag="em")
                st = bpool.tile([128, slots_b], BF16, tag="st")
                rT = bpool.tile([128, slots_b], BF16, tag="rT")
                sidx_t = bpool.tile([128, slots_b // 16], I16, tag="sidx")
                recvb_t = bpool.tile([128, slots_b // 128], BF16, tag="recvb")
                nodesb_t = bpool.tile([66, B * 128], F32, tag="nodesb")

                nc.sync.dma_start(sidx_t[:],
                                  sidx.ap()[:, s0 // 16:(s0 + slots_b) // 16])
                nc.sync.dma_start(recvb_t[:],
                                  recvb.ap()[:, b * B * tiles_w:(b + 1) * B * tiles_w])
                nc.sync.dma_start(nodesb_t[:],
                                  nodes_t.ap()[:, b * B * 128:(b + 1) * B * 128])
                # partition-broadcast of window-local receiver ids
                nc.sync.dma_start(
                    rT[:], recvf.ap()[0:1, s0:s0 + slots_b].to_broadcast(
                        [128, slots_b]))

                # sender gathers: one per quarter, spread over 4 SWDGE queues
                qs = B * slots_wq                     # slots per quarter in batch
                for q in range(4):
                    nc.gpsimd.dma_gather(
                        out_ap=em[:, q * qs:(q + 1) * qs]
                        .rearrange("p (c f) -> p c f", f=128),
                        in_ap=tbl_s.ap()[q * QUARTER:(q + 1) * QUARTER, :],
                        idxs_ap=sidx_t[:, q * qs // 16:(q + 1) * qs // 16],
                        num_idxs=qs, num_idxs_reg=qs, elem_size=128,
                        transpose=False, single_packet=False, queue_num=q,
                    )
                # batched per-128-block transpose: edge-major -> feature-major
                nc.sync.dma_start(
                    out=st[:].rearrange("p (k f) -> p k f", f=128),
                    in_=em[:].rearrange("p (k f) -> p k f", f=128),
                    transpose=True)
                # edge features + ones row then overwrite spare partitions 64:98
                nc.sync.dma_start(st[64:98, :], edges_t.ap()[:, s0:s0 + slots_b])

                for wi in range(B):
                    wg = b * B + wi                   # global window index
                    # receiver projection for this window's 128 nodes
                    prw_ps = pprw.tile([128, HIDDEN], F32, tag="prw")
                    nc.tensor.matmul(
                        out=prw_ps[:],
                        lhsT=nodesb_t[0:NODE_F, wi * 128:(wi + 1) * 128],
                        rhs=w1r_t[:], start=True, stop=True)
                    prw_s = spool.tile([128, HIDDEN], BF16, tag="prw_s")
                    nc.scalar.activation(prw_s[:], prw_ps[:], cpy)

                    agg_ps = pagg.tile([128, 128], F32, tag="agg")
                    for q in range(4):
                        # quarter-block of tiles_wq tiles (slots_wq slots)
                        off = q * qs + wi * slots_wq
                        tcol = off // 128
                        mask_b = spool.tile([128, slots_wq], BF16, tag="mask")
                        nc.vector.tensor_tensor(
                            out=mask_b[:].rearrange("p (c f) -> p c f", f=128),
                            in0=recvb_t[:, tcol:tcol + tiles_wq]
                            .to_broadcast([128, tiles_wq, 128]),
                            in1=iota_t[:].rearrange("p (c f) -> p c f", f=128),
                            op=iseq)
                        maskT_b = spool.tile([128, slots_wq], BF16, tag="maskT")
                        nc.vector.tensor_scalar(
                            out=maskT_b[:], in0=rT[:, off:off + slots_wq],
                            scalar1=iotap_t[:], scalar2=None, op0=iseq)
                        h_ps = ph.tile([128, slots_wq], F32, tag="h")
                        for j in range(tiles_wq):
                            so = off + j * 128
                            hsl = h_ps[:, j * 128:(j + 1) * 128]
                            nc.tensor.matmul(out=hsl, lhsT=st[:, so:so + 128],
                                             rhs=w1mod_t[:], start=True,
                                             stop=False)
                            nc.tensor.matmul(
                                out=hsl,
                                lhsT=maskT_b[:, j * 128:(j + 1) * 128],
                                rhs=prw_s[:], start=False, stop=True)
                        h_s = spool.tile([128, slots_wq], BF16, tag="hs")
                        nc.scalar.activation(h_s[:], h_ps[:], relu)
                        for j in range(tiles_wq):
                            nc.tensor.matmul(
                                out=agg_ps[:],
                                lhsT=h_s[:, j * 128:(j + 1) * 128],
                                rhs=mask_b[:, j * 128:(j + 1) * 128],
                                start=(q == 0 and j == 0),
                                stop=(q == 3 and j == tiles_wq - 1))
                    # window epilogue: out = (aggT.T@W2)*invdeg + nodes@waug
                    agg_s = opool.tile([128, 128], BF16, tag="aggs")
                    nc.scalar.activation(agg_s[:], agg_ps[:], cpy)
                    ot_ps = pout.tile([128, 2 * OUT_F], F32, tag="ot")
                    nc.tensor.matmul(out=ot_ps[:, 0:OUT_F], lhsT=agg_s[:],
                                     rhs=w2_t[:], start=True, stop=True)
                    nc.tensor.matmul(out=ot_ps[:, OUT_F:2 * OUT_F],
                                     lhsT=nodesb_t[:, wi * 128:(wi + 1) * 128],
                                     rhs=waug_t[:], start=True, stop=True)
                    t1 = opool.tile([128, OUT_F], F32, tag="t1")
                    nc.vector.tensor_scalar(
                        out=t1[:], in0=ot_ps[:, 0:OUT_F],
                        scalar1=invdeg_t[:, wg:wg + 1],
                        scalar2=None, op0=mybir.AluOpType.mult)
                    ot = opool.tile([128, OUT_F], F32, tag="otf")
                    nc.vector.tensor_add(ot[:], t1[:], ot_ps[:, OUT_F:2 * OUT_F])
                    nc.sync.dma_start(out_d.ap()[wg * 128:(wg + 1) * 128, :], ot[:])

    nc.compile()
    return nc


def _prep_inputs(nodes, edges, senders, receivers, W1, b1, W2, b2, Wn, bn,
                 slots_wq):
    """Host-side data layout. Returns per-core in_maps."""
    slots_w = 4 * slots_wq
    slots_core = W_CORE * slots_w

    nodes_pad = np.zeros((NODES_PAD, NODE_F), np.float32)
    nodes_pad[:N_NODES] = nodes

    tbl_s = np.zeros((NODES_PAD, 128), bfnp)
    tbl_s[:, :NODE_F] = nodes_pad.astype(bfnp)

    deg = np.bincount(receivers, minlength=NODES_PAD).astype(np.float32)
    invdeg_full = 1.0 / np.maximum(deg, 1.0)
    gate_full = (deg > 0).astype(np.float32)

    # shared weight tensors
    w1mod = np.zeros((128, HIDDEN), bfnp)
    w1mod[:NODE_F] = W1[:NODE_F].astype(bfnp)
    w1mod[NODE_F:NODE_F + EDGE_F] = W1[2 * NODE_F:].astype(bfnp)
    w1mod[NODE_F + EDGE_F] = b1.astype(bfnp)
    w1r = np.ascontiguousarray(W1[NODE_F:2 * NODE_F]).astype(np.float32)
    w2b = W2.astype(bfnp)
    waug = np.zeros((66, OUT_F), np.float32)
    waug[:NODE_F] = Wn
    waug[NODE_F] = b2
    waug[NODE_F + 1] = bn
    iota_b = np.tile(np.arange(128, dtype=np.float32), (128, slots_wq // 128)
                     ).astype(bfnp)
    iotap = np.arange(128, dtype=np.float32).reshape(128, 1)
    iotap8 = np.arange(128, dtype=np.uint8).reshape(128, 1)

    core_of_edge = receivers // NODES_CORE
    in_maps = []
    for c in range(NCORES):
        lo = c * NODES_CORE
        eid = np.nonzero(core_of_edge == c)[0]
        rloc = receivers[eid] - lo
        w_loc = rloc >> 7
        q = senders[eid] // QUARTER
        # sender as fastest key: each (window, quarter) group's gather reads
        # the node table in ascending order (HBM row-buffer locality)
        order = np.lexsort((senders[eid], q, w_loc))
        eid, rloc, w_loc, q = eid[order], rloc[order], w_loc[order], q[order]
        grp = w_loc * 4 + q
        counts = np.bincount(grp, minlength=W_CORE * 4)
        assert counts.max() <= slots_wq, f"quarter run {counts.max()} > {slots_wq}"
        starts = np.searchsorted(grp, np.arange(W_CORE * 4))
        pos = np.arange(len(eid)) - starts[grp]
        base_wq = ((w_loc // B) * (B * slots_w) + q * (B * slots_wq)
                   + (w_loc % B) * slots_wq)
        slot = base_wq + pos

        sidx_f = np.zeros(slots_core, np.int16)
        sidx_f[slot] = (senders[eid] % QUARTER).astype(np.int16)
        recvw = np.full(slots_core, 200.0, np.float32)
        recvw[slot] = (rloc & 127).astype(np.float32)
        edges_t = np.zeros((34, slots_core), bfnp)
        edges_t[:EDGE_F, slot] = edges[eid].T.astype(bfnp)
        edges_t[EDGE_F, slot] = 1.0

        nodes_taug = np.zeros((66, NODES_CORE), np.float32)
        nodes_taug[:NODE_F] = nodes_pad[lo:lo + NODES_CORE].T
        nodes_taug[NODE_F] = gate_full[lo:lo + NODES_CORE]
        nodes_taug[NODE_F + 1] = 1.0

        in_maps.append({
            "tbl_s": tbl_s,
            "edges_t": edges_t,
            "sidx": np.tile(sidx_f.reshape(-1, 16).T, (8, 1)),
            "recvb": recvw.astype(bfnp).reshape(-1, 128).T.copy(),
            "recvf": recvw.astype(bfnp).reshape(1, -1),
            "nodes_t": nodes_taug,
            "invdeg": invdeg_full[lo:lo + NODES_CORE].reshape(-1, 128).T.copy(),
            "w1mod": w1mod, "w1r": w1r, "w2": w2b, "waug": waug,
            "iota": iota_b, "iotap": iotap, "iotap8": iotap8,
        })
    return in_maps


def kernel(nodes, edges, senders, receivers, W1, b1, W2, b2, Wn, bn,
           _trace=False):
    senders = np.asarray(senders).astype(np.int64)
    receivers = np.asarray(receivers).astype(np.int64)
    nodes = np.asarray(nodes, np.float32)
    edges = np.asarray(edges, np.float32)

    # fixed quarter-run capacity; recompile only if data exceeds it
    slots_wq = 640
    cnt = np.bincount(
        (receivers // NODES_CORE) * (W_CORE * 4)
        + (((receivers % NODES_CORE) >> 7) * 4) + senders // QUARTER,
        minlength=NCORES * W_CORE * 4).max()
    while cnt > slots_wq:
        slots_wq += 128

    if slots_wq not in _cache:
        _cache[slots_wq] = _build_program(slots_wq)
    nc = _cache[slots_wq]

    in_maps = _prep_inputs(nodes, edges, senders, receivers,
                           np.asarray(W1, np.float32), np.asarray(b1, np.float32),
                           np.asarray(W2, np.float32), np.asarray(b2, np.float32),
                           np.asarray(Wn, np.float32), np.asarray(bn, np.float32),
                           slots_wq)

    res = bass_utils.run_bass_kernel_spmd(
        nc, in_maps, core_ids=list(range(NCORES)), trace=_trace)

    out = np.concatenate([res.results[c]["out"] for c in range(NCORES)], axis=0)
    kernel.last_results = res
    return out[:N_NODES]
